# revision 2
# baseline (speedup 1.0000x reference)
"""Self-contained Trainium2 Bass kernel for nn_Denoiser_77841987273333 (v3).

kernel(**inputs) takes the FULL inputs and returns the FULL [4, 8192, 3]
output. Shards batch*half across 8 NeuronCores (core = 2*b + half; each core
handles 4096 query rows of one batch against the full 8192-point candidate
set), compiles one SPMD Bass program, runs it on cores 0-7, reassembles.

Design:
  - z' = -d^2/2 - 0.125 as ONE 13-row split-bf16 matmul (hi/lo keeps ~1e-4
    abs accuracy at bf16 speed, 1 cyc/col).
  - top-16 per row WITHOUT materializing z in SBUF: per-1024-chunk max8 +
    max_index directly on PSUM, then a tiny global phase on 64 candidates
    with packed (quantized value << 13 | 8191-index) uint32 keys compared
    as fp32 bit patterns; bitwise decode gives exact indices + tie-breaks.
  - on-chip index plumbing: PE transpose + 8 SBUF->SBUF DMA copies.
  - neighbor features recomputed from gathered bf16 coords; diff-term
    folded into conv weights (Wc1r = a+c, Wc1k = b-c); f-MLP's first
    layer fused into the conv matmul (stacked [3,128] weights).
  - score dot-products: elementwise muls + partition all-reduce on GPSIMD.
  - 4-stage software pipeline per 128-row tile: scans(t) | conv/scores(t-1)
    | softmax(t-2) | output(t-3), so each engine stream stays dense.
"""
from contextlib import ExitStack

import ml_dtypes
import numpy as np

import concourse.bass as bass
import concourse.bass_isa as bass_isa
import concourse.mybir as mybir
import concourse.tile as tile
from concourse.bass_utils import run_bass_kernel_spmd
from concourse.masks import make_identity

F32 = mybir.dt.float32
BF16 = mybir.dt.bfloat16
U16 = mybir.dt.uint16
U32 = mybir.dt.uint32
I32 = mybir.dt.int32
AF = mybir.ActivationFunctionType
ALU = mybir.AluOpType
LRELU = 0.01

B, N, NQ, K = 4, 8192, 4096, 16
N_CORES = 8
NT = NQ // 128          # 32 row-tiles
NCH = 8                 # z chunks per tile
CH = N // NCH           # 1024 cand per chunk
GC = 128 * K            # 2048 gathered cols per tile
NCC = GC // 512         # conv chunks

WNAMES = ["WcfT", "W2Tp", "Wc1r", "Wc1s", "Wc2T", "Wc3", "WqW2T", "WqC3T",
          "Wk0", "Wk1", "W1T", "ones128"]
WSHAPES = [[3, 128], [128, 128], [3, 64], [3, 64], [64, 64], [128, 64],
           [64, 256], [64, 256], [128, 256], [128, 256], [3, 64], [128, 1]]
BNAMES = ["bcf", "b2c", "bc1c", "bc2c", "bqf0", "bqf1", "b1c"]
BSHAPES = [[128, 1], [128, 1], [64, 1], [64, 1], [128, 1], [128, 1],
           [64, 1]]


def build(nc: bass.Bass):
    lz_d = nc.dram_tensor("lz", [13, NQ], BF16, kind="ExternalInput")
    rz_d = nc.dram_tensor("rz", [13, N], BF16, kind="ExternalInput")
    xg_d = nc.dram_tensor("xg", [128, N], BF16, kind="ExternalInput")
    xqb_d = nc.dram_tensor("xqb", [3, NQ], BF16, kind="ExternalInput")
    wd = {m: nc.dram_tensor(m, s, BF16, kind="ExternalInput")
          for m, s in zip(WNAMES, WSHAPES)}
    bd = {m: nc.dram_tensor(m, s, F32, kind="ExternalInput")
          for m, s in zip(BNAMES, BSHAPES)}
    srep_d = nc.dram_tensor("SrepT", [16, 128], F32, kind="ExternalInput")
    out_d = nc.dram_tensor("out", [3, NQ], F32, kind="ExternalOutput")
    s_dram = nc.dram_tensor("s_scratch", [2, GC], F32, kind="Internal")
    w_dram = nc.dram_tensor("w_scratch", [2, 128, K], BF16, kind="Internal")

    with tile.TileContext(nc) as tc, ExitStack() as ctx:
        const = ctx.enter_context(tc.tile_pool(name="const", bufs=1))
        lz = const.tile([13, NQ], BF16)
        rz = const.tile([13, N], BF16)
        xg = const.tile([128, N], BF16)
        xqb = const.tile([3, NQ], BF16)
        W = {m: const.tile(s, BF16, name=m, tag=m) for m, s in zip(WNAMES, WSHAPES)}
        Bi = {m: const.tile(s, F32, name=m, tag=m) for m, s in zip(BNAMES, BSHAPES)}
        identf = const.tile([128, 128], F32)
        srep = const.tile([16, 128], F32)
        nbig = const.tile([128, 128], F32)
        co = const.tile([128, 64], I32)
        s13 = const.tile([128, 1], U32)
        s3ff = const.tile([128, 1], U32)

        for t, dd in [(lz, lz_d), (rz, rz_d), (xqb, xqb_d)]:
            nc.sync.dma_start(out=t[:], in_=dd[:])
        for xc in range(8):
            xsl = bass.ts(xc, N // 8)
            nc.sync.dma_start(out=xg[:, xsl], in_=xg_d[:, xsl])
        for m in WNAMES:
            nc.sync.dma_start(out=W[m][:], in_=wd[m][:])
        for m in BNAMES:
            nc.sync.dma_start(out=Bi[m][:], in_=bd[m][:])
        nc.sync.dma_start(out=srep[:], in_=srep_d[:])
        make_identity(nc, identf[:])
        make_identity(nc, nbig[:])
        nc.scalar.mul(out=nbig[:], in_=nbig[:], mul=-1.0e30)
        nc.gpsimd.iota(co[:], [[-CH, NCH], [0, 8]], base=CH * (NCH - 1),
                       channel_multiplier=0)
        nc.vector.memset(s13[:], 13)
        nc.vector.memset(s3ff[:], 0x3FF)
        u0a = const.tile([128, NQ], BF16)
        va = const.tile([64, NQ], BF16)

        zps = ctx.enter_context(tc.tile_pool(name="zps", bufs=2, space="PSUM"))
        wps = ctx.enter_context(tc.tile_pool(name="wps", bufs=3, space="PSUM"))
        tps = ctx.enter_context(tc.tile_pool(name="tps", bufs=1, space="PSUM"))
        fp = ctx.enter_context(tc.tile_pool(name="front", bufs=2))
        gp = ctx.enter_context(tc.tile_pool(name="gat", bufs=4))
        bp = ctx.enter_context(tc.tile_pool(name="back", bufs=3))
        sp = ctx.enter_context(tc.tile_pool(name="small", bufs=4))

        # ---------- batched self-attention precompute (all rows) ----------
        for cc in range(NQ // 512):
            sl = bass.ts(cc, 512)
            p1 = wps.tile([128, 512], F32, tag="w")
            nc.tensor.matmul(p1[0:64, :], W["Wc1s"][:], xqb[:, sl],
                             start=True, stop=True)
            h1s = sp.tile([64, 512], BF16, tag="h1s")
            nc.scalar.activation(h1s[:], p1[0:64, :], AF.Lrelu,
                                 bias=Bi["bc1c"][:], alpha=LRELU)
            p2 = wps.tile([128, 512], F32, tag="w")
            nc.tensor.matmul(p2[0:64, :], W["Wc2T"][:], h1s[:],
                             start=True, stop=True)
            h2s = sp.tile([64, 512], BF16, tag="h2s")
            nc.scalar.activation(h2s[:], p2[0:64, :], AF.Lrelu,
                                 bias=Bi["bc2c"][:], alpha=LRELU)
            pf = wps.tile([128, 512], F32, tag="w")
            nc.tensor.matmul(pf[0:64, :], W["W1T"][:], xqb[:, sl],
                             start=True, stop=True)
            f1s = sp.tile([64, 512], BF16, tag="f1s")
            nc.scalar.activation(f1s[:], pf[0:64, :], AF.Relu,
                                 bias=Bi["b1c"][:])
            q01 = sp.tile([128, 1024], BF16, tag="q01")
            for h, bqn in enumerate(["bqf0", "bqf1"]):
                qp = wps.tile([128, 512], F32, tag="w")
                nc.tensor.matmul(qp[:], W["WqW2T"][:, bass.ts(h, 128)],
                                 f1s[:], start=True, stop=False)
                nc.tensor.matmul(qp[:], W["WqC3T"][:, bass.ts(h, 128)],
                                 h2s[:], start=False, stop=True)
                nc.scalar.activation(q01[:, bass.ts(h, 512)], qp[:],
                                     AF.Identity, bias=Bi[bqn][:])
            up0 = wps.tile([128, 512], F32, tag="w")
            nc.tensor.matmul(up0[:], W["Wk0"][:, 0:128], q01[:, 0:512],
                             start=True, stop=False)
            nc.tensor.matmul(up0[:], W["Wk1"][:, 0:128], q01[:, 512:1024],
                             start=False, stop=True)
            nc.scalar.activation(u0a[:, sl], up0[:], AF.Copy)
            up1 = wps.tile([128, 512], F32, tag="w")
            nc.tensor.matmul(up1[:], W["Wk0"][:, 128:256], q01[:, 0:512],
                             start=True, stop=False)
            nc.tensor.matmul(up1[:], W["Wk1"][:, 128:256], q01[:, 512:1024],
                             start=False, stop=True)
            u1 = sp.tile([128, 512], BF16, tag="u1")
            nc.scalar.activation(u1[:], up1[:], AF.Copy)
            vp = wps.tile([128, 512], F32, tag="w")
            nc.tensor.matmul(vp[0:64, :], W["Wc3"][:], u1[:], start=True,
                             stop=True)
            nc.scalar.activation(va[:, sl], vp[0:64, :], AF.Copy)

        # ---------- stage F (tile t): z chunks + scans ----------
        def front(t, cv, ci):
            trows = bass.ds(t * 128, 128)
            for c in range(NCH):
                zp = zps.tile([128, CH], F32, tag="z")
                for h in range(CH // 512):
                    nc.tensor.matmul(zp[:, bass.ts(h, 512)], lz[:, trows],
                                     rz[:, bass.ds(c * CH + h * 512, 512)],
                                     start=True, stop=True)
                if c == (t * 128) // CH:
                    o = (t * 128) % CH
                    nc.vector.tensor_add(zp[:, bass.ds(o, 128)],
                                         zp[:, bass.ds(o, 128)], nbig[:])
                nc.vector.max(out=cv[:, bass.ts(c, 8)], in_=zp[:])
                nc.vector.max_index(out=ci[:, bass.ts(c, 8)],
                                    in_max=cv[:, bass.ts(c, 8)], in_values=zp[:])

        # ---------- stage F-tail (tile t): pack, select, idx, gather -------
        def front_tail(t, cv, ci):
            k1 = fp.tile([128, 64], U32, tag="k1")
            nc.vector.tensor_scalar(k1[:], cv[:], 6.35, scalar2=32768.0,
                                    op0=ALU.add, op1=ALU.mult)
            enc = fp.tile([128, 64], U32, tag="enc")
            nc.vector.scalar_tensor_tensor(enc[:], ci[:], s3ff[:, 0:1],
                                           co[:].bitcast(U32),
                                           op0=ALU.bitwise_xor,
                                           op1=ALU.bitwise_or)
            k3 = fp.tile([128, 64], F32, tag="k3")
            nc.vector.scalar_tensor_tensor(k3[:].bitcast(U32), k1[:],
                                           s13[:, 0:1], enc[:],
                                           op0=ALU.logical_shift_left,
                                           op1=ALU.bitwise_or)
            g1 = fp.tile([128, 8], F32, tag="g1")
            g2 = fp.tile([128, 8], F32, tag="g2")
            nc.vector.max(out=g1[:], in_=k3[:])
            nc.vector.match_replace(out=k3[:], in_to_replace=g1[:],
                                    in_values=k3[:], imm_value=0.0)
            nc.vector.max(out=g2[:], in_=k3[:])
            id16 = fp.tile([128, K], U32, tag="id16")
            nc.vector.tensor_scalar(id16[:, 0:8], g1[:].bitcast(U32), 0x1FFF,
                                    scalar2=0x1FFF, op0=ALU.bitwise_and,
                                    op1=ALU.bitwise_xor)
            nc.vector.tensor_scalar(id16[:, 8:16], g2[:].bitcast(U32), 0x1FFF,
                                    scalar2=0x1FFF, op0=ALU.bitwise_and,
                                    op1=ALU.bitwise_xor)
            idf = fp.tile([128, K], F32, tag="idf")
            nc.vector.tensor_copy(idf[:], id16[:])
            tpp = tps.tile([16, 128], F32, tag="tp")
            nc.tensor.transpose(tpp[:], idf[:], identf[:])
            idxTf = fp.tile([16, 128], F32, tag="idxTf")
            nc.scalar.activation(idxTf[:], tpp[:], AF.Copy)
            rp = wps.tile([128, 512], F32, tag="w")
            nc.tensor.matmul(rp[:, 0:128], srep[:], idxTf[:], start=True,
                             stop=True)
            idxw = fp.tile([128, 128], U16, tag="idxw")
            nc.scalar.activation(idxw[:], rp[:, 0:128], AF.Copy)
            ka = gp.tile([128, GC], BF16, tag="ka")
            for h in range(2):
                nc.gpsimd.indirect_copy(ka[:, bass.ts(h, 1024)], xg[:],
                                        idxw[:, bass.ts(h, 64)], True)
            return ka

        # ---------- stage A (tile t): conv + attention + scores ----------
        def back_a(t, ka, st):
            trows = bass.ds(t * 128, 128)
            h2 = bp.tile([64, GC], BF16, tag="h2")
            kf = bp.tile([128, GC], BF16, tag="kf")
            hg = sp.tile([128, 512], BF16, tag="hg")
            for c in range(NCC):
                sl = bass.ts(c, 512)
                rep_c = xqb[:, bass.ds(t * 128 + c * 32, 32)].to_broadcast(
                    [3, 32, K])
                p1 = wps.tile([128, 512], F32, tag="w")
                nc.tensor.matmul(p1[:], W["WcfT"][:], ka[0:3, sl],
                                 start=True, stop=False)
                nc.tensor.matmul(p1[0:64, :], W["Wc1r"][:], rep_c,
                                 start=False, stop=True, skip_group_check=True)
                nc.scalar.activation(hg[0:64, :], p1[0:64, :], AF.Lrelu,
                                     bias=Bi["bcf"][0:64], alpha=LRELU)
                nc.scalar.activation(hg[64:128, :], p1[64:128, :], AF.Relu,
                                     bias=Bi["bcf"][64:128])
                p2 = wps.tile([128, 512], F32, tag="w")
                nc.tensor.matmul(p2[0:64, :], W["Wc2T"][:], hg[0:64, :],
                                 start=True, stop=True)
                nc.scalar.activation(h2[:, sl], p2[0:64, :], AF.Lrelu,
                                     bias=Bi["bc2c"][:], alpha=LRELU)
                pf2 = wps.tile([128, 512], F32, tag="w")
                nc.tensor.matmul(pf2[:], W["W2Tp"][64:128, :], hg[64:128, :],
                                 start=True, stop=True)
                nc.scalar.activation(kf[:, sl], pf2[:], AF.Identity,
                                     bias=Bi["b2c"][:])
            # scores: elementwise muls + fold h2v into kfv (pool), then
            # a single ones-matmul partition-sum per 512 chunk (PE)
            u0b = u0a[:, trows].to_broadcast([128, 128, K])
            kfv = kf[:].rearrange("c (r j) -> c r j", j=K)
            nc.gpsimd.tensor_mul(kfv, kfv, u0b)
            vb = va[:, trows].to_broadcast([64, 128, K])
            h2v = h2[:, 0:GC].rearrange("c (r j) -> c r j", j=K)
            nc.gpsimd.tensor_mul(h2v, h2v, vb)
            nc.gpsimd.tensor_add(kf[0:64, :], kf[0:64, :], h2[0:64, :])
            s = bp.tile([1, GC], F32, tag="s", name="s")
            for c in range(NCC):
                sl = bass.ts(c, 512)
                spp = wps.tile([128, 512], F32, tag="w")
                nc.tensor.matmul(spp[0:1, :], W["ones128"][:], kf[:, sl],
                                 start=True, stop=True)
                nc.scalar.activation(s[0:1, sl], spp[0:1, :], AF.Copy)
            nc.sync.dma_start(out=s_dram[t % 2][None, :], in_=s[0:1, :])
            nc.sync.dma_start(
                out=st[:, 0:K],
                in_=s_dram[t % 2][None, :].rearrange("o (r j) -> (o r) j",
                                                     j=K))

        # ---------- stage B (tile t): softmax + weight plumbing ----------
        def back_b(t, st):
            stf = sp.tile([128, 4], F32, tag="stf")
            nc.vector.tensor_reduce(stf[:, 0:1], st[:, 0:K],
                                    axis=mybir.AxisListType.X,
                                    op=mybir.AluOpType.max, negate=True)
            e = sp.tile([128, K], BF16, tag="e")
            nc.scalar.activation(e[:, 0:K], st[:, 0:K], AF.Exp,
                                 bias=stf[:, 0:1])
            nc.vector.tensor_reduce(stf[:, 1:2], e[:, 0:K],
                                    axis=mybir.AxisListType.X,
                                    op=mybir.AluOpType.add)
            nc.vector.reciprocal(stf[:, 2:3], stf[:, 1:2])
            w16 = sp.tile([128, K], BF16, tag="w16")
            nc.vector.tensor_scalar_mul(w16[:], e[:, 0:K], stf[:, 2:3])
            nc.sync.dma_start(out=w_dram[t % 2], in_=w16[:])
            w3 = bp.tile([3, GC], BF16, tag="w3", name="w3")
            nc.sync.dma_start(
                out=w3[:],
                in_=w_dram[t % 2].rearrange("r j -> (r j)")[None, :]
                .broadcast_to([3, GC]))
            return w3

        # ---------- stage C (tile t): weighted output ----------
        def back_c(t, ka, w3):
            nc.vector.tensor_mul(ka[0:3, :], ka[0:3, :], w3[:])
            nx = sp.tile([16, 128], F32, tag="nx")
            nc.vector.tensor_reduce(nx[0:3, :],
                                    ka[0:3, :].rearrange("c (r j) -> c r j",
                                                         j=K),
                                    axis=mybir.AxisListType.X,
                                    op=mybir.AluOpType.add)
            nc.sync.dma_start(out=out_d[:, bass.ds(t * 128, 128)],
                              in_=nx[0:3, :])

        # ---------- software-pipelined main loop ----------
        ka_h = {}   # t -> ka tile
        st_h = {}   # t -> st tile
        w3_h = {}   # t -> w3 tile
        for t in range(NT + 3):
            if t < NT:
                cv = fp.tile([128, 64], F32, tag="cv")
                ci = fp.tile([128, 64], U32, tag="ci")
                front(t, cv, ci)
                ka_h[t] = front_tail(t, cv, ci)
            if t - 1 >= 0 and t - 1 < NT:
                st_h[t - 1] = sp.tile([128, K], F32, tag="st", name="st")
                back_a(t - 1, ka_h[t - 1], st_h[t - 1])
            if t - 2 >= 0 and t - 2 < NT:
                w3_h[t - 2] = back_b(t - 2, st_h.pop(t - 2))
            if t - 3 >= 0:
                back_c(t - 3, ka_h.pop(t - 3), w3_h.pop(t - 3))
    return nc


def prep_weights(w: dict):
    bf = ml_dtypes.bfloat16
    f32 = lambda x: np.ascontiguousarray(np.asarray(x, np.float32))
    Wc1T = f32(w["Wc1"]).T
    a, b, c = Wc1T[0:3], Wc1T[3:6], Wc1T[6:9]
    W1T = f32(w["W1"]).T           # [3, 64]
    W2T = f32(w["W2"]).T           # [64, 128]
    WcfT = np.concatenate([b - c, W1T], axis=1)       # [3, 128]
    W2Tp = np.zeros((128, 128), np.float32)
    W2Tp[64:128] = W2T
    Wq = f32(w["Wq"])              # [256, 256]
    WqW2 = Wq[:, 0:128] @ f32(w["W2"])     # [256, 64]
    WqC3 = Wq[:, 128:256] @ f32(w["Wc3"])  # [256, 64]
    bqf = (f32(w["bq"]) + Wq[:, 0:128] @ f32(w["b2"])
           + Wq[:, 128:256] @ f32(w["bc3"]))
    wb = {
        "WcfT": WcfT, "W2Tp": W2Tp, "Wc1r": a + c, "Wc1s": a + b,
        "Wc2T": f32(w["Wc2"]).T, "Wc3": f32(w["Wc3"]),
        "WqW2T": WqW2.T, "WqC3T": WqC3.T,
        "Wk0": f32(w["Wk"])[0:128], "Wk1": f32(w["Wk"])[128:256],
        "W1T": W1T, "ones128": np.ones((128, 1), np.float32),
    }
    out = {k: np.ascontiguousarray(v).astype(bf) for k, v in wb.items()}
    bcf = np.concatenate([f32(w["bc1"]), f32(w["b1"])])[:, None]
    out["SrepT"] = np.ascontiguousarray(np.tile(np.eye(16, dtype=np.float32), 8))
    out.update({
        "bcf": np.ascontiguousarray(bcf),
        "b1c": f32(w["b1"])[:, None], "b2c": f32(w["b2"])[:, None],
        "bc1c": f32(w["bc1"])[:, None], "bc2c": f32(w["bc2"])[:, None],
        "bqf0": np.ascontiguousarray(bqf[0:128, None]),
        "bqf1": np.ascontiguousarray(bqf[128:256, None]),
    })
    return out


def prep_core(x_b: np.ndarray, r0: int):
    bf = ml_dtypes.bfloat16
    xr = np.roll(np.asarray(x_b, np.float32), -r0, axis=0)   # [N, 3]
    xq = xr[0:NQ]
    n_j = -0.5 * (xr * xr).sum(-1)
    m_i = -0.5 * (xq * xq).sum(-1) - 0.125

    def sp(aa):
        hi = aa.astype(bf).astype(np.float32)
        lo = (aa - hi).astype(bf).astype(np.float32)
        return hi, lo

    xh, xl = sp(xr.T)          # [3, N]
    qh, ql = sp(xq.T)          # [3, NQ]
    nh, nl = sp(n_j)
    mh, ml_ = sp(m_i)
    rz = np.zeros((13, N), np.float32)
    rz[0:3] = xh; rz[3:6] = xh; rz[6:9] = xl
    rz[9] = nh; rz[10] = nl; rz[11] = 1.0; rz[12] = 1.0
    lzm = np.zeros((13, NQ), np.float32)
    lzm[0:3] = qh; lzm[3:6] = ql; lzm[6:9] = qh
    lzm[9] = 1.0; lzm[10] = 1.0; lzm[11] = mh; lzm[12] = ml_
    xgm = np.zeros((128, N), np.float32)
    for g in range(8):
        xgm[16 * g:16 * g + 3] = xh
    return {
        "rz": rz.astype(bf), "lz": lzm.astype(bf),
        "xg": xgm.astype(bf), "xqb": qh.astype(bf),
    }


# ---------------------------------------------------------------------------
# Sync legalizer: the walrus in this container encodes at most ~2 sync
# commands per instruction; Tile emits up to 12 inline waits. Split excess
# waits into standalone EventSemaphore instructions.
# ---------------------------------------------------------------------------
import json as _json

import concourse.bass2jax as _bass2jax
import concourse.bass_utils as _bass_utils


def _legalize_sync(bir_json):
    d = _json.loads(bir_json)
    for fn in d["functions"]:
        for bb in fn["blocks"]:
            out = []
            for inst in bb["instructions"]:
                si = inst.get("sync_info")
                waits = (si or {}).get("on_wait") or []
                budget = 1
                if len(waits) > budget:
                    split, keep = waits[:-budget], waits[-budget:]
                    for i, w in enumerate(split):
                        out.append({
                            "debug": inst.get("debug", 0),
                            "engine": inst["engine"],
                            "ins": [], "outs": [],
                            "name": f"{inst['name']}-sw{i}",
                            "opcode": "EventSemaphore",
                            "sync_info": {"on_update": [], "on_wait": [w]},
                        })
                    si["on_wait"] = keep
                out.append(inst)
            bb["instructions"] = out
    return _json.dumps(d).encode()


_orig_compile_bir_kernel = _bass_utils.compile_bir_kernel


def _patched_compile_bir_kernel(bir_json, tmpdir, neff_name="file.neff"):
    return _orig_compile_bir_kernel(_legalize_sync(bir_json), tmpdir,
                                    neff_name=neff_name)


if _bass_utils.compile_bir_kernel is not _patched_compile_bir_kernel:
    _bass_utils.compile_bir_kernel = _patched_compile_bir_kernel
    _bass2jax.compile_bir_kernel = _patched_compile_bir_kernel


_CACHE = {}


def _get_nc():
    if "nc" not in _CACHE:
        nc = bass.Bass("TRN2")
        build(nc)
        _CACHE["nc"] = nc
    return _CACHE["nc"]


def kernel(x, global_feat, W1, b1, W2, b2, Wc1, bc1, Wc2, bc2, Wc3, bc3,
           Wq, bq, Wk, bk, _profile=None):
    del global_feat  # unused by the reference forward
    x = np.asarray(x, np.float32)
    w = prep_weights(dict(W1=W1, b1=b1, W2=W2, b2=b2, Wc1=Wc1, bc1=bc1,
                          Wc2=Wc2, bc2=bc2, Wc3=Wc3, bc3=bc3, Wq=Wq, bq=bq,
                          Wk=Wk, bk=bk))
    in_maps = []
    for core in range(N_CORES):
        b, half = core // 2, core % 2
        m = dict(w)
        m.update(prep_core(x[b], half * NQ))
        in_maps.append(m)

    nc = _get_nc()
    kwargs = dict(_profile) if _profile else {}
    res = run_bass_kernel_spmd(nc, in_maps, core_ids=list(range(N_CORES)),
                               **kwargs)
    out = np.zeros((B, N, 3), np.float32)
    for core in range(N_CORES):
        b, half = core // 2, core % 2
        out[b, half * NQ:(half + 1) * NQ] = res.results[core]["out"].T
    if _profile is not None and isinstance(_profile, dict):
        _profile["exec_time_ns"] = res.exec_time_ns
    return out
